# revision 4
# baseline (speedup 1.0000x reference)
"""Antialiased bicubic 4x downscale (blur -> bicubic/2, twice) on 8 TRN2 cores.

The whole chain is linear and separable: every stage is M_H (x) M_W acting on
the H/W axes, so the composition collapses to a single 1024->256 banded matrix
T applied on both sides: out = T @ X @ T^T per (batch, channel) image.

Sharding: pure data parallel - batch 16 -> 2 per core, 6 images/core.

The kernel is HBM-bound: 24 MiB of x per core streams at line rate (~356 GB/s
measured) on the gpsimd SWDGE ring, in [p, c, w] layout (4 KB descriptors).
All compute (pass 1 f32r matmuls exploiting T's band sparsity, PE transposes,
bf16 pass 2) hides under the stream.  The ends are trimmed:
  leader: image 0's first 1 MiB chunk rides the sync HWDGE ring (fast first
    byte), the rest of the stream follows on gpsimd.
  tail: the last image arrives as column chunks (512/384/128 cols).  Z columns
    0-207 depend only on the first two chunks (T is banded) and are finalized
    and stored while the last chunk streams.  The final 128-col chunk skips
    the pass1->evac->transpose chain: Yt[qc=7] is computed directly with the
    x slices as stationary (out = x_chunk^T Tt-block), leaving only 2 small
    pass-2 matmuls and a 48-column z store after the last byte.
"""

import numpy as np
import ml_dtypes

import concourse.bacc as bacc
import concourse.mybir as mybir
import concourse.tile as tile
from concourse.bass_utils import run_bass_kernel_spmd

SIGMA = 0.66
BICUBIC_W = np.array([-0.09375, 0.59375, 0.59375, -0.09375], dtype=np.float64)

N_CORES = 8
B, C, H, W = 16, 3, 1024, 1024
HO = H // 4
IMGS = (B // N_CORES) * C  # 6 images per core

F32 = mybir.dt.float32
F32R = mybir.dt.float32r
BF16 = mybir.dt.bfloat16

# z column split for the last image: z[:, 0:ZCUT] depends only on x cols
# 0-895 (qc 0-6); z[:, ZCUT:] gets qc 6 and 7 contributions.
ZCUT = 208


def _gauss_matrix(n):
    x = np.arange(3, dtype=np.float32) - np.float32(1.0)
    k = np.exp(np.float32(-0.5) * (x / np.float32(SIGMA)) ** 2)
    k = (k / k.sum()).astype(np.float64)
    G = np.zeros((n, n))
    for t in range(3):
        G += k[t] * np.eye(n, n, t - 1)
    return G


def _down_matrix(n):
    # out[i] = sum_t w[t] * x[clamp(2i + t - 1, 0, n-1)]
    m = n // 2
    D = np.zeros((m, n))
    for i in range(m):
        for t in range(4):
            j = min(max(2 * i + t - 1, 0), n - 1)
            D[i, j] += BICUBIC_W[t]
    return D


def build_T():
    T = _down_matrix(H // 2) @ _gauss_matrix(H // 2) @ _down_matrix(H) @ _gauss_matrix(H)
    return T.astype(np.float32)  # [256, 1024]


def _pass1_pieces(Tt):
    """(pc, ih) pairs where Tt[128pc:128pc+128, 128ih:128ih+128] is nonzero."""
    pieces = []
    for ih in range(2):
        for pc in range(8):
            if np.any(Tt[128 * pc : 128 * (pc + 1), 128 * ih : 128 * (ih + 1)]):
                pieces.append((pc, ih))
    return pieces


def _build_graph():
    Tt = build_T().T  # [1024, 256]
    pieces = _pass1_pieces(Tt)
    pcs_by_ih = [[pc for (pc, ih2) in pieces if ih2 == ih] for ih in range(2)]

    nc = bacc.Bacc("TRN2", target_bir_lowering=False, debug=False)
    x = nc.dram_tensor("x", [IMGS, H, W], F32R, kind="ExternalInput").ap()
    # tb is host-prearranged to the SBUF layout: tb[p, c, n] = Tt[128c+p, n]
    tb = nc.dram_tensor("tb", [128, 8, HO], BF16, kind="ExternalInput").ap()
    eye = nc.dram_tensor("eye", [128, 128], BF16, kind="ExternalInput").ap()
    # out in SBUF layout [p, img, c, j] = Z[img, 128c+p, j]; host unscrambles
    out = nc.dram_tensor("out", [128, IMGS, 2, HO], F32, kind="ExternalOutput").ap()

    with tile.TileContext(nc) as tc:
        with (
            tc.tile_pool(name="const", bufs=1) as cpool,
            tc.tile_pool(name="xin", bufs=3) as xpool,
            tc.tile_pool(name="ysb", bufs=2) as ypool,
            tc.tile_pool(name="ytsb", bufs=2) as ytpool,
            tc.tile_pool(name="zout", bufs=2) as zpool,
            tc.tile_pool(name="psy", bufs=4, space="PSUM") as psy,
            tc.tile_pool(name="pst", bufs=2, space="PSUM") as pst,
            tc.tile_pool(name="ps2", bufs=2, space="PSUM") as ps2,
        ):
            ttb = cpool.tile([128, 8, HO], BF16, tag="ttb")
            nc.scalar.dma_start(out=ttb[:], in_=tb)
            ident = cpool.tile([128, 128], BF16, tag="ident")
            nc.scalar.dma_start(out=ident[:], in_=eye)
            # f32r copy of Tt for pass 1 stationary, cast on-chip
            tt = cpool.tile([128, 8, HO], F32R, tag="tt")
            nc.vector.tensor_copy(tt[:], ttb[:])

            def p1mm(yq, pc, ih, xap, start, stop):
                nc.tensor.matmul(
                    yq,
                    tt[:, pc, 128 * ih : 128 * (ih + 1)],
                    xap,
                    start=start,
                    stop=stop,
                )

            for img in range(IMGS):
                xt = xpool.tile([128, 8, W], F32R, tag="xt", name=f"xt{img}")
                xr = x[img].rearrange("(c p) w -> p c w", p=128)

                y_sb = ypool.tile([128, 2, W], BF16)
                yt_sb = ytpool.tile([128, 8, HO], BF16)
                z = zpool.tile([128, 2, HO], F32, tag="zout", name=f"z{img}")

                def evac(dst, src, ih):
                    if ih == 0:
                        nc.vector.tensor_copy(dst, src)
                    else:
                        nc.scalar.copy(dst, src)

                def transposes(ih, qc0, nqc, tag):
                    tp = pst.tile(
                        [128, 128 * nqc], BF16, tag="pst",
                        name=f"tp{img}_{tag}_{ih}",
                    )
                    for s in range(nqc):
                        qc = qc0 + s
                        nc.tensor.matmul(
                            tp[:, 128 * s : 128 * (s + 1)],
                            y_sb[:, ih, 128 * qc : 128 * (qc + 1)],
                            ident[:],
                            is_transpose=True,
                            start=(s == 0),
                            stop=(s == nqc - 1),
                        )
                    dst = yt_sb[:, qc0 : qc0 + nqc, 128 * ih : 128 * (ih + 1)]
                    tsrc = tp[:].rearrange("p (s w) -> p s w", s=nqc)
                    evac(dst, tsrc, ih)

                def p2mm(acc, qc, ih, jslice, start, stop):
                    nc.tensor.matmul(
                        acc,
                        yt_sb[:, qc, 128 * ih : 128 * (ih + 1)],
                        ttb[:, qc, jslice],
                        start=start,
                        stop=stop,
                    )

                if img < IMGS - 1:
                    # row-block chunked loads; 4 KB descriptors
                    if img == 0:
                        # first chunk on the sync HWDGE ring: fast first byte
                        nc.sync.dma_start(out=xt[:, 0:2], in_=xr[:, 0:2])
                        nc.gpsimd.dma_start(out=xt[:, 2:8], in_=xr[:, 2:8])
                    else:
                        nc.gpsimd.dma_start(out=xt[:, 0:4], in_=xr[:, 0:4])
                        nc.gpsimd.dma_start(out=xt[:, 4:8], in_=xr[:, 4:8])
                    for ch in range(2):
                        for ih in range(2):
                            yq = psy.tile(
                                [128, 512], F32, tag="psy",
                                name=f"psy{img}_{ch}_{ih}",
                            )
                            pcs = pcs_by_ih[ih]
                            for k, pc in enumerate(pcs):
                                p1mm(yq[:], pc, ih,
                                     xt[:, pc, 512 * ch : 512 * (ch + 1)],
                                     k == 0, k == len(pcs) - 1)
                            evac(y_sb[:, ih, 512 * ch : 512 * (ch + 1)], yq[:], ih)
                        for ih in range(2):
                            transposes(ih, 4 * ch, 4, f"c{ch}")
                    for ih in range(2):
                        acc = ps2.tile([128, HO], F32, tag="ps2",
                                       name=f"ps2_{img}_{ih}")
                        for qc in range(8):
                            p2mm(acc[:], qc, ih, slice(0, HO), qc == 0, qc == 7)
                        evac(z[:, ih, :], acc[:], ih)
                    nc.sync.dma_start(out=out[:, img], in_=z[:])
                else:
                    # last image: column chunks A (qc 0-3), B (qc 4-6),
                    # C (qc 7, split in two row halves)
                    nc.gpsimd.dma_start(out=xt[:, :, 0:512], in_=xr[:, :, 0:512])
                    nc.gpsimd.dma_start(out=xt[:, :, 512:896], in_=xr[:, :, 512:896])
                    nc.gpsimd.dma_start(out=xt[:, 0:4, 896:1024],
                                        in_=xr[:, 0:4, 896:1024])
                    nc.gpsimd.dma_start(out=xt[:, 4:8, 896:1024],
                                        in_=xr[:, 4:8, 896:1024])

                    # chunk A: classic pass 1 on cols 0-511
                    for ih in range(2):
                        yq = psy.tile([128, 512], F32, tag="psy",
                                      name=f"psyA_{ih}")
                        pcs = pcs_by_ih[ih]
                        for k, pc in enumerate(pcs):
                            p1mm(yq[:], pc, ih, xt[:, pc, 0:512],
                                 k == 0, k == len(pcs) - 1)
                        evac(y_sb[:, ih, 0:512], yq[:], ih)
                    for ih in range(2):
                        transposes(ih, 0, 4, "A")
                    # chunk B: classic pass 1 on cols 512-895
                    for ih in range(2):
                        yq = psy.tile([128, 512], F32, tag="psy",
                                      name=f"psyB_{ih}")
                        pcs = pcs_by_ih[ih]
                        for k, pc in enumerate(pcs):
                            p1mm(yq[:, 0:384], pc, ih, xt[:, pc, 512:896],
                                 k == 0, k == len(pcs) - 1)
                        evac(y_sb[:, ih, 512:896], yq[:, 0:384], ih)
                    for ih in range(2):
                        transposes(ih, 4, 3, "B")
                    # zA = z[:, :, 0:ZCUT]: needs qc 0-6 only
                    for ih in range(2):
                        acc = ps2.tile([128, HO], F32, tag="ps2",
                                       name=f"ps2A_{ih}")
                        for qc in range(7):
                            p2mm(acc[:, 0:ZCUT], qc, ih, slice(0, ZCUT),
                                 qc == 0, qc == 6)
                        evac(z[:, ih, 0:ZCUT], acc[:, 0:ZCUT], ih)
                    nc.sync.dma_start(out=out[:, img, :, 0:ZCUT],
                                      in_=z[:, :, 0:ZCUT])

                    # chunk C (cols 896-1023): Yt[qc=7] directly, x as
                    # stationary: ytc = sum_pc x[:, pc, 896:]^T @ Tt[pc-block]
                    ytc = ps2.tile([128, HO], F32, tag="ps2", name="ytc")
                    for pc in range(8):
                        nc.tensor.matmul(
                            ytc[:],
                            xt[:, pc, 896:1024],
                            tt[:, pc, :],
                            start=(pc == 0),
                            stop=(pc == 7),
                        )
                    nc.vector.tensor_copy(yt_sb[:, 7, :], ytc[:])
                    # zB = z[:, :, ZCUT:]: qc 6 and 7 contributions
                    for ih in range(2):
                        acc = ps2.tile([128, HO], F32, tag="ps2",
                                       name=f"ps2B_{ih}")
                        p2mm(acc[:, 0 : HO - ZCUT], 6, ih, slice(ZCUT, HO),
                             True, False)
                        p2mm(acc[:, 0 : HO - ZCUT], 7, ih, slice(ZCUT, HO),
                             False, True)
                        evac(z[:, ih, ZCUT:HO], acc[:, 0 : HO - ZCUT], ih)
                    nc.sync.dma_start(out=out[:, img, :, ZCUT:HO],
                                      in_=z[:, :, ZCUT:HO])
    nc.compile()
    return nc


_GRAPH = None


def _get_graph():
    global _GRAPH
    if _GRAPH is None:
        _GRAPH = _build_graph()
    return _GRAPH


def run(x, **spmd_kwargs):
    x = np.ascontiguousarray(np.asarray(x, dtype=np.float32))
    assert x.shape == (B, C, H, W)
    nc = _get_graph()
    Tt = build_T().T  # [1024, 256] f32
    tb_host = np.ascontiguousarray(
        Tt.reshape(8, 128, HO).transpose(1, 0, 2)
    ).astype(ml_dtypes.bfloat16)
    eye_host = np.eye(128, dtype=ml_dtypes.bfloat16)
    per_core = B // N_CORES
    in_maps = [
        {
            "x": x[i * per_core : (i + 1) * per_core].reshape(IMGS, H, W),
            "tb": tb_host,
            "eye": eye_host,
        }
        for i in range(N_CORES)
    ]
    res = run_bass_kernel_spmd(nc, in_maps, core_ids=list(range(N_CORES)), **spmd_kwargs)
    outs = []
    for r in res.results:
        o = r["out"].transpose(1, 2, 0, 3).reshape(IMGS, 2 * 128, HO)
        outs.append(o.reshape(per_core, C, HO, HO))
    return np.concatenate(outs, axis=0), res


def kernel(x):
    out, _ = run(x)
    return out


# revision 5
# speedup vs baseline: 1.1142x; 1.1142x over previous
"""Antialiased bicubic 4x downscale (blur -> bicubic/2, twice) on 8 TRN2 cores.

The whole chain is linear and separable: every stage is M_H (x) M_W acting on
the H/W axes, so the composition collapses to a single 1024->256 banded matrix
T applied on both sides: out = T @ X @ T^T per (batch, channel) image.

Sharding: pure data parallel - batch 16 -> 2 per core, 6 images/core.

The kernel is HBM-bound: 24 MiB of x per core streams at line rate (~356 GB/s
measured) on the gpsimd SWDGE ring; descriptors are kept >= 2 KB (smaller
runs measurably collapse the end-of-stream drain rate).  All compute (pass 1
f32r matmuls exploiting T's band sparsity, PE transposes, bf16 pass 2) hides
under the stream; the work that remains after the last byte is minimized:

  The last image arrives as ch0 (cols 0-511), then cols 512-1023 in three
  row-block groups.  Columns 512-1023 are processed via a transposed pass 1
  (Yt[qc] accumulated directly with the arriving x row-blocks as stationary,
  mov = banded Tt windows), so no evac+PE-transpose chain trails the stream.
  The output is split at column 126: z[:, 0:126] depends only on qc 0-3
  (= ch0) and is stored while cols 512+ stream; only the final row group's
  8 windowed matmuls, one cast, 8 small pass-2 matmuls and a 130-column
  store follow the last byte.
"""

import numpy as np
import ml_dtypes

import concourse.bacc as bacc
import concourse.mybir as mybir
import concourse.tile as tile
from concourse.bass_utils import run_bass_kernel_spmd

SIGMA = 0.66
BICUBIC_W = np.array([-0.09375, 0.59375, 0.59375, -0.09375], dtype=np.float64)

N_CORES = 8
B, C, H, W = 16, 3, 1024, 1024
HO = H // 4
IMGS = (B // N_CORES) * C  # 6 images per core

F32 = mybir.dt.float32
F32R = mybir.dt.float32r
BF16 = mybir.dt.bfloat16

# z column split for the last image: z[:, 0:ZCUT] depends only on qc 0-3
# (x cols 0-511); z[:, ZCUT:] gets qc 3-7 contributions.
ZCUT = 126


def _gauss_matrix(n):
    x = np.arange(3, dtype=np.float32) - np.float32(1.0)
    k = np.exp(np.float32(-0.5) * (x / np.float32(SIGMA)) ** 2)
    k = (k / k.sum()).astype(np.float64)
    G = np.zeros((n, n))
    for t in range(3):
        G += k[t] * np.eye(n, n, t - 1)
    return G


def _down_matrix(n):
    # out[i] = sum_t w[t] * x[clamp(2i + t - 1, 0, n-1)]
    m = n // 2
    D = np.zeros((m, n))
    for i in range(m):
        for t in range(4):
            j = min(max(2 * i + t - 1, 0), n - 1)
            D[i, j] += BICUBIC_W[t]
    return D


def build_T():
    T = _down_matrix(H // 2) @ _gauss_matrix(H // 2) @ _down_matrix(H) @ _gauss_matrix(H)
    return T.astype(np.float32)  # [256, 1024]


def _pass1_pieces(Tt):
    """(pc, ih) pairs where Tt[128pc:128pc+128, 128ih:128ih+128] is nonzero."""
    pieces = []
    for ih in range(2):
        for pc in range(8):
            if np.any(Tt[128 * pc : 128 * (pc + 1), 128 * ih : 128 * (ih + 1)]):
                pieces.append((pc, ih))
    return pieces


def _pc_windows(Tt):
    """Per row-block pc, the [a, b) span of nonzero columns of Tt."""
    wins = []
    for pc in range(8):
        nz = np.nonzero(np.any(Tt[128 * pc : 128 * (pc + 1), :] != 0, axis=0))[0]
        wins.append((int(nz.min()), int(nz.max()) + 1))
    return wins


def _build_graph():
    Tt = build_T().T  # [1024, 256]
    pieces = _pass1_pieces(Tt)
    pcs_by_ih = [[pc for (pc, ih2) in pieces if ih2 == ih] for ih in range(2)]
    wins = _pc_windows(Tt)

    nc = bacc.Bacc("TRN2", target_bir_lowering=False, debug=False)
    x = nc.dram_tensor("x", [IMGS, H, W], F32R, kind="ExternalInput").ap()
    # tb is host-prearranged to the SBUF layout: tb[p, c, n] = Tt[128c+p, n]
    tb = nc.dram_tensor("tb", [128, 8, HO], BF16, kind="ExternalInput").ap()
    eye = nc.dram_tensor("eye", [128, 128], BF16, kind="ExternalInput").ap()
    # out in SBUF layout [p, img, c, j] = Z[img, 128c+p, j]; host unscrambles
    out = nc.dram_tensor("out", [128, IMGS, 2, HO], F32, kind="ExternalOutput").ap()

    with tile.TileContext(nc) as tc:
        with (
            tc.tile_pool(name="const", bufs=1) as cpool,
            tc.tile_pool(name="xin", bufs=3) as xpool,
            tc.tile_pool(name="ysb", bufs=2) as ypool,
            tc.tile_pool(name="ytsb", bufs=2) as ytpool,
            tc.tile_pool(name="zout", bufs=2) as zpool,
            tc.tile_pool(name="psy", bufs=3, space="PSUM") as psy,
            tc.tile_pool(name="pst", bufs=1, space="PSUM") as pst,
            tc.tile_pool(name="ps2", bufs=2, space="PSUM") as ps2,
            tc.tile_pool(name="ptq", bufs=2, space="PSUM") as ptq,
        ):
            # tiny warmup load: spins up the SWDGE queue/engines so the real
            # stream's first bytes land sooner
            warm = cpool.tile([128, 8], F32R, tag="warm")
            nc.gpsimd.dma_start(out=warm[:], in_=x[0, 0:128, 0:8])

            ttb = cpool.tile([128, 8, HO], BF16, tag="ttb")
            nc.scalar.dma_start(out=ttb[:], in_=tb)
            ident = cpool.tile([128, 128], BF16, tag="ident")
            nc.scalar.dma_start(out=ident[:], in_=eye)
            # f32r copy of Tt for pass 1 stationary, cast on-chip
            tt = cpool.tile([128, 8, HO], F32R, tag="tt")
            nc.vector.tensor_copy(tt[:], ttb[:])

            def p1mm(yq, pc, ih, xap, start, stop):
                nc.tensor.matmul(
                    yq,
                    tt[:, pc, 128 * ih : 128 * (ih + 1)],
                    xap,
                    start=start,
                    stop=stop,
                )

            for img in range(IMGS):
                xt = xpool.tile([128, 8, W], F32R, tag="xt", name=f"xt{img}")
                xr = x[img].rearrange("(c p) w -> p c w", p=128)

                y_sb = ypool.tile([128, 2, W], BF16)
                yt_sb = ytpool.tile([128, 8, HO], BF16)
                z = zpool.tile([128, 2, HO], F32, tag="zout", name=f"z{img}")

                def evac(dst, src, ih):
                    if ih == 0:
                        nc.vector.tensor_copy(dst, src)
                    else:
                        nc.scalar.copy(dst, src)

                def transposes(ih, qc0, nqc, tag):
                    tp = pst.tile(
                        [128, 512], BF16, tag="pst",
                        name=f"tp{img}_{tag}_{ih}",
                    )
                    for s in range(nqc):
                        qc = qc0 + s
                        nc.tensor.matmul(
                            tp[:, 128 * s : 128 * (s + 1)],
                            y_sb[:, ih, 128 * qc : 128 * (qc + 1)],
                            ident[:],
                            is_transpose=True,
                            start=(s == 0),
                            stop=(s == nqc - 1),
                        )
                    dst = yt_sb[:, qc0 : qc0 + nqc, 128 * ih : 128 * (ih + 1)]
                    tsrc = tp[:, 0 : 128 * nqc].rearrange("p (s w) -> p s w", s=nqc)
                    evac(dst, tsrc, ih)

                def p2mm(acc, qc, ih, jslice, start, stop):
                    nc.tensor.matmul(
                        acc,
                        yt_sb[:, qc, 128 * ih : 128 * (ih + 1)],
                        ttb[:, qc, jslice],
                        start=start,
                        stop=stop,
                    )

                if img < IMGS - 1:
                    # row-block chunked loads; 4 KB descriptors
                    nc.gpsimd.dma_start(out=xt[:, 0:4], in_=xr[:, 0:4])
                    nc.gpsimd.dma_start(out=xt[:, 4:8], in_=xr[:, 4:8])
                    for ch in range(2):
                        for ih in range(2):
                            yq = psy.tile(
                                [128, 512], F32, tag="psy",
                                name=f"psy{img}_{ch}_{ih}",
                            )
                            pcs = pcs_by_ih[ih]
                            for k, pc in enumerate(pcs):
                                p1mm(yq[:], pc, ih,
                                     xt[:, pc, 512 * ch : 512 * (ch + 1)],
                                     k == 0, k == len(pcs) - 1)
                            evac(y_sb[:, ih, 512 * ch : 512 * (ch + 1)], yq[:], ih)
                        for ih in range(2):
                            transposes(ih, 4 * ch, 4, f"c{ch}")
                    for ih in range(2):
                        acc = ps2.tile([128, HO], F32, tag="ps2",
                                       name=f"ps2_{img}_{ih}")
                        for qc in range(8):
                            p2mm(acc[:], qc, ih, slice(0, HO), qc == 0, qc == 7)
                        evac(z[:, ih, :], acc[:], ih)
                    nc.sync.dma_start(out=out[:, img], in_=z[:])
                else:
                    # last image: ch0 (cols 0-511), then cols 512-1023 in
                    # three row-block groups (2 KB descriptors throughout)
                    nc.gpsimd.dma_start(out=xt[:, :, 0:512], in_=xr[:, :, 0:512])
                    nc.gpsimd.dma_start(out=xt[:, 0:4, 512:1024],
                                        in_=xr[:, 0:4, 512:1024])
                    nc.gpsimd.dma_start(out=xt[:, 4:6, 512:1024],
                                        in_=xr[:, 4:6, 512:1024])
                    nc.gpsimd.dma_start(out=xt[:, 6:8, 512:1024],
                                        in_=xr[:, 6:8, 512:1024])

                    # ch0: classic pass 1 + transposes -> yt qc 0-3
                    for ih in range(2):
                        yq = psy.tile([128, 512], F32, tag="psy",
                                      name=f"psyL_{ih}")
                        pcs = pcs_by_ih[ih]
                        for k, pc in enumerate(pcs):
                            p1mm(yq[:], pc, ih, xt[:, pc, 0:512],
                                 k == 0, k == len(pcs) - 1)
                        evac(y_sb[:, ih, 0:512], yq[:], ih)
                    for ih in range(2):
                        transposes(ih, 0, 4, "L")

                    # zA = z[:, :, 0:ZCUT]: qc 0-3 only; store early
                    for ih in range(2):
                        acc = ps2.tile([128, ZCUT], F32, tag="ps2",
                                       name=f"ps2A_{ih}")
                        for qc in range(4):
                            p2mm(acc[:], qc, ih, slice(0, ZCUT),
                                 qc == 0, qc == 3)
                        evac(z[:, ih, 0:ZCUT], acc[:], ih)
                    nc.sync.dma_start(out=out[:, img, :, 0:ZCUT],
                                      in_=z[:, :, 0:ZCUT])

                    # zB starts with its qc3 contribution (available early)
                    zb = [
                        ps2.tile([128, HO - ZCUT], F32, tag="ps2",
                                 name=f"ps2B_{ih}")
                        for ih in range(2)
                    ]
                    for ih in range(2):
                        p2mm(zb[ih][:], 3, ih, slice(ZCUT, HO), True, False)

                    # cols 512-1023 via transposed pass 1: Yt[qc] accumulated
                    # with x row-blocks as stationary, banded mov windows.
                    # ytq tiles: [qc4,qc5] and [qc6,qc7], one PSUM bank each;
                    # one accumulation group per bank (start on first MM,
                    # stop on last).
                    ytq = [
                        ptq.tile([128, 2, HO], F32, tag="ytq",
                                 name=f"ytq{g}")
                        for g in range(2)
                    ]
                    for pcg in ((0, 1, 2, 3), (4, 5), (6, 7)):
                        for pc in pcg:
                            a, b = wins[pc]
                            for qc in (4, 5, 6, 7):
                                g, s = divmod(qc - 4, 2)
                                nc.tensor.matmul(
                                    ytq[g][:, s, a:b],
                                    xt[:, pc, 128 * qc : 128 * (qc + 1)],
                                    tt[:, pc, a:b],
                                    start=(pc == 0 and s == 0),
                                    stop=(pc == 7 and s == 1),
                                )
                    nc.vector.tensor_copy(yt_sb[:, 4:6, :], ytq[0][:])
                    nc.scalar.copy(yt_sb[:, 6:8, :], ytq[1][:])

                    # zB: qc 4-7 contributions, evac, final store
                    for ih in range(2):
                        for qc in range(4, 8):
                            p2mm(zb[ih][:], qc, ih, slice(ZCUT, HO),
                                 False, qc == 7)
                        evac(z[:, ih, ZCUT:HO], zb[ih][:], ih)
                    nc.sync.dma_start(out=out[:, img, :, ZCUT:HO],
                                      in_=z[:, :, ZCUT:HO])
    nc.compile()
    return nc


_GRAPH = None


def _get_graph():
    global _GRAPH
    if _GRAPH is None:
        _GRAPH = _build_graph()
    return _GRAPH


def run(x, **spmd_kwargs):
    x = np.ascontiguousarray(np.asarray(x, dtype=np.float32))
    assert x.shape == (B, C, H, W)
    nc = _get_graph()
    Tt = build_T().T  # [1024, 256] f32
    tb_host = np.ascontiguousarray(
        Tt.reshape(8, 128, HO).transpose(1, 0, 2)
    ).astype(ml_dtypes.bfloat16)
    eye_host = np.eye(128, dtype=ml_dtypes.bfloat16)
    per_core = B // N_CORES
    in_maps = [
        {
            "x": x[i * per_core : (i + 1) * per_core].reshape(IMGS, H, W),
            "tb": tb_host,
            "eye": eye_host,
        }
        for i in range(N_CORES)
    ]
    res = run_bass_kernel_spmd(nc, in_maps, core_ids=list(range(N_CORES)), **spmd_kwargs)
    outs = []
    for r in res.results:
        o = r["out"].transpose(1, 2, 0, 3).reshape(IMGS, 2 * 128, HO)
        outs.append(o.reshape(per_core, C, HO, HO))
    return np.concatenate(outs, axis=0), res


def kernel(x):
    out, _ = run(x)
    return out


# revision 6
# speedup vs baseline: 1.1420x; 1.0249x over previous
"""Antialiased bicubic 4x downscale (blur -> bicubic/2, twice) on 8 TRN2 cores.

The whole chain is linear and separable: every stage is M_H (x) M_W acting on
the H/W axes, so the composition collapses to a single 1024->256 banded matrix
T applied on both sides: out = T @ X @ T^T per (batch, channel) image.

Sharding: pure data parallel - batch 16 -> 2 per core, 6 images/core.

The kernel is HBM-bound: 24 MiB of x per core streams at line rate (~360 GB/s
measured) on the gpsimd SWDGE ring; descriptors are kept >= 2 KB (smaller
runs measurably collapse the end-of-stream drain rate).  A tiny warmup DMA
absorbs the DMA-queue spin-up before the stream.  All compute (pass 1 f32r
matmuls exploiting T's band sparsity, PE transposes, bf16 pass 2) hides under
the stream; the work remaining after the last byte is minimized:

  The last image arrives as ch0 (cols 0-511, processed classically early),
  then cols 512-1023 in three row-block groups, cast to bf16 in-flight
  (SWDGE).  Those columns take a transposed pass 1: Yt[qc 4-7] accumulate
  directly with the arriving bf16 x row-blocks as stationary (fast weight
  load) against banded Tt windows - no evac+PE-transpose chain trails the
  stream.  The output is split at column 126: z[:, 0:126] (plus the full
  qc 0-3 partial for the rest) is computed and stored while cols 512+
  stream; after the last byte only 8 windowed matmuls, two casts, 8 small
  pass-2 matmuls, two fused add-evacs and a 130-column store remain.
"""

import numpy as np
import ml_dtypes

import concourse.bacc as bacc
import concourse.mybir as mybir
import concourse.tile as tile
from concourse.bass_utils import run_bass_kernel_spmd

SIGMA = 0.66
BICUBIC_W = np.array([-0.09375, 0.59375, 0.59375, -0.09375], dtype=np.float64)

N_CORES = 8
B, C, H, W = 16, 3, 1024, 1024
HO = H // 4
IMGS = (B // N_CORES) * C  # 6 images per core

F32 = mybir.dt.float32
F32R = mybir.dt.float32r
BF16 = mybir.dt.bfloat16

# z column split for the last image: z[:, 0:ZCUT] depends only on qc 0-3
# (x cols 0-511) and is stored early; z[:, ZCUT:] additionally gets the
# qc 4-7 contributions added after the stream ends.
ZCUT = 126


def _gauss_matrix(n):
    x = np.arange(3, dtype=np.float32) - np.float32(1.0)
    k = np.exp(np.float32(-0.5) * (x / np.float32(SIGMA)) ** 2)
    k = (k / k.sum()).astype(np.float64)
    G = np.zeros((n, n))
    for t in range(3):
        G += k[t] * np.eye(n, n, t - 1)
    return G


def _down_matrix(n):
    # out[i] = sum_t w[t] * x[clamp(2i + t - 1, 0, n-1)]
    m = n // 2
    D = np.zeros((m, n))
    for i in range(m):
        for t in range(4):
            j = min(max(2 * i + t - 1, 0), n - 1)
            D[i, j] += BICUBIC_W[t]
    return D


def build_T():
    T = _down_matrix(H // 2) @ _gauss_matrix(H // 2) @ _down_matrix(H) @ _gauss_matrix(H)
    return T.astype(np.float32)  # [256, 1024]


def _pass1_pieces(Tt):
    """(pc, ih) pairs where Tt[128pc:128pc+128, 128ih:128ih+128] is nonzero."""
    pieces = []
    for ih in range(2):
        for pc in range(8):
            if np.any(Tt[128 * pc : 128 * (pc + 1), 128 * ih : 128 * (ih + 1)]):
                pieces.append((pc, ih))
    return pieces


def _pc_windows(Tt):
    """Per row-block pc, the [a, b) span of nonzero columns of Tt."""
    wins = []
    for pc in range(8):
        nz = np.nonzero(np.any(Tt[128 * pc : 128 * (pc + 1), :] != 0, axis=0))[0]
        wins.append((int(nz.min()), int(nz.max()) + 1))
    return wins


def _build_graph():
    Tt = build_T().T  # [1024, 256]
    pieces = _pass1_pieces(Tt)
    pcs_by_ih = [[pc for (pc, ih2) in pieces if ih2 == ih] for ih in range(2)]
    wins = _pc_windows(Tt)

    nc = bacc.Bacc("TRN2", target_bir_lowering=False, debug=False)
    x = nc.dram_tensor("x", [IMGS, H, W], F32R, kind="ExternalInput").ap()
    # tb is host-prearranged to the SBUF layout: tb[p, c, n] = Tt[128c+p, n]
    tb = nc.dram_tensor("tb", [128, 8, HO], BF16, kind="ExternalInput").ap()
    eye = nc.dram_tensor("eye", [128, 128], BF16, kind="ExternalInput").ap()
    # out in SBUF layout [p, img, c, j] = Z[img, 128c+p, j]; host unscrambles
    out = nc.dram_tensor("out", [128, IMGS, 2, HO], F32, kind="ExternalOutput").ap()

    with tile.TileContext(nc) as tc:
        with (
            tc.tile_pool(name="const", bufs=1) as cpool,
            tc.tile_pool(name="xin", bufs=3) as xpool,
            tc.tile_pool(name="ysb", bufs=2) as ypool,
            tc.tile_pool(name="ytsb", bufs=2) as ytpool,
            tc.tile_pool(name="zout", bufs=2) as zpool,
            tc.tile_pool(name="psy", bufs=4, space="PSUM") as psy,
            tc.tile_pool(name="pst", bufs=2, space="PSUM") as pst,
            tc.tile_pool(name="ps2", bufs=2, space="PSUM") as ps2,
        ):
            # tiny warmup load: spins up the SWDGE queue/engines so the real
            # stream's first bytes land sooner
            warm = cpool.tile([128, 8], F32R, tag="warm")
            nc.gpsimd.dma_start(out=warm[:], in_=x[0, 0:128, 0:8])

            ttb = cpool.tile([128, 8, HO], BF16, tag="ttb")
            nc.scalar.dma_start(out=ttb[:], in_=tb)
            ident = cpool.tile([128, 128], BF16, tag="ident")
            nc.scalar.dma_start(out=ident[:], in_=eye)
            # f32r copy of Tt for pass 1 stationary, cast on-chip
            tt = cpool.tile([128, 8, HO], F32R, tag="tt")
            nc.vector.tensor_copy(tt[:], ttb[:])

            def p1mm(yq, pc, ih, xap, start, stop):
                nc.tensor.matmul(
                    yq,
                    tt[:, pc, 128 * ih : 128 * (ih + 1)],
                    xap,
                    start=start,
                    stop=stop,
                )

            for img in range(IMGS):
                xt = xpool.tile([128, 8, W], F32R, tag="xt", name=f"xt{img}")
                xr = x[img].rearrange("(c p) w -> p c w", p=128)

                y_sb = ypool.tile([128, 2, W], BF16)
                yt_sb = ytpool.tile([128, 8, HO], BF16)
                z = zpool.tile([128, 2, HO], F32, tag="zout", name=f"z{img}")

                def evac(dst, src, ih):
                    if ih == 0:
                        nc.vector.tensor_copy(dst, src)
                    else:
                        nc.scalar.copy(dst, src)

                def transposes(ih, qc0, nqc, tag):
                    tp = pst.tile(
                        [128, 512], BF16, tag="pst",
                        name=f"tp{img}_{tag}_{ih}",
                    )
                    for s in range(nqc):
                        qc = qc0 + s
                        nc.tensor.matmul(
                            tp[:, 128 * s : 128 * (s + 1)],
                            y_sb[:, ih, 128 * qc : 128 * (qc + 1)],
                            ident[:],
                            is_transpose=True,
                            start=(s == 0),
                            stop=(s == nqc - 1),
                        )
                    dst = yt_sb[:, qc0 : qc0 + nqc, 128 * ih : 128 * (ih + 1)]
                    tsrc = tp[:, 0 : 128 * nqc].rearrange("p (s w) -> p s w", s=nqc)
                    evac(dst, tsrc, ih)

                def p2mm(acc, qc, ih, jslice, start, stop):
                    nc.tensor.matmul(
                        acc,
                        yt_sb[:, qc, 128 * ih : 128 * (ih + 1)],
                        ttb[:, qc, jslice],
                        start=start,
                        stop=stop,
                    )

                if img < IMGS - 1:
                    # row-block chunked loads; 4 KB descriptors
                    nc.gpsimd.dma_start(out=xt[:, 0:4], in_=xr[:, 0:4])
                    nc.gpsimd.dma_start(out=xt[:, 4:8], in_=xr[:, 4:8])
                    for ch in range(2):
                        for ih in range(2):
                            yq = psy.tile(
                                [128, 512], F32, tag="psy",
                                name=f"psy{img}_{ch}_{ih}",
                            )
                            pcs = pcs_by_ih[ih]
                            for k, pc in enumerate(pcs):
                                p1mm(yq[:], pc, ih,
                                     xt[:, pc, 512 * ch : 512 * (ch + 1)],
                                     k == 0, k == len(pcs) - 1)
                            evac(y_sb[:, ih, 512 * ch : 512 * (ch + 1)], yq[:], ih)
                        for ih in range(2):
                            transposes(ih, 4 * ch, 4, f"c{ch}")
                    for ih in range(2):
                        acc = ps2.tile([128, HO], F32, tag="ps2",
                                       name=f"ps2_{img}_{ih}")
                        for qc in range(8):
                            p2mm(acc[:], qc, ih, slice(0, HO), qc == 0, qc == 7)
                        evac(z[:, ih, :], acc[:], ih)
                    nc.sync.dma_start(out=out[:, img], in_=z[:])
                else:
                    # last image: ch0 (cols 0-511, f32r), then cols 512-1023
                    # in three row-block groups cast to bf16 in-flight
                    # (2 KB source descriptors throughout)
                    xb = xpool.tile([128, 8, 512], BF16, tag="xb")
                    xrb = x[img].bitcast(F32).rearrange("(c p) w -> p c w", p=128)
                    nc.gpsimd.dma_start(out=xt[:, :, 0:512], in_=xr[:, :, 0:512])
                    nc.gpsimd.dma_start(out=xb[:, 0:4], in_=xrb[:, 0:4, 512:1024])
                    nc.gpsimd.dma_start(out=xb[:, 4:6], in_=xrb[:, 4:6, 512:1024])
                    nc.gpsimd.dma_start(out=xb[:, 6:8], in_=xrb[:, 6:8, 512:1024])

                    # ch0: classic pass 1 + transposes -> yt qc 0-3
                    for ih in range(2):
                        yq = psy.tile([128, 512], F32, tag="psy",
                                      name=f"psyL_{ih}")
                        pcs = pcs_by_ih[ih]
                        for k, pc in enumerate(pcs):
                            p1mm(yq[:], pc, ih, xt[:, pc, 0:512],
                                 k == 0, k == len(pcs) - 1)
                        evac(y_sb[:, ih, 0:512], yq[:], ih)
                    for ih in range(2):
                        transposes(ih, 0, 4, "L")

                    # zA: full-width qc 0-3 partial; store cols 0-125 early
                    # (they are complete), keep the rest in z for the late add
                    for ih in range(2):
                        acc = ps2.tile([128, HO], F32, tag="ps2",
                                       name=f"ps2A_{ih}")
                        for qc in range(4):
                            p2mm(acc[:], qc, ih, slice(0, HO), qc == 0, qc == 3)
                        evac(z[:, ih, :], acc[:], ih)
                    nc.sync.dma_start(out=out[:, img, :, 0:ZCUT],
                                      in_=z[:, :, 0:ZCUT])

                    # cols 512-1023 via transposed pass 1: Yt[qc 4-7]
                    # accumulated with bf16 x row-blocks as stationary (FWL),
                    # banded mov windows; one accumulation group per PSUM
                    # bank (start on its first MM, stop on its last)
                    ytq = [
                        ps2.tile([128, 2, HO], F32, tag="ps2",
                                 name=f"ytq{g}")
                        for g in range(2)
                    ]
                    for pcg in ((0, 1, 2, 3), (4, 5), (6, 7)):
                        for pc in pcg:
                            a, b = wins[pc]
                            for qc in (4, 5, 6, 7):
                                g, s = divmod(qc - 4, 2)
                                nc.tensor.matmul(
                                    ytq[g][:, s, a:b],
                                    xb[:, pc, 128 * (qc - 4) : 128 * (qc - 3)],
                                    ttb[:, pc, a:b],
                                    start=(pc == 0 and s == 0),
                                    stop=(pc == 7 and s == 1),
                                )
                    nc.vector.tensor_copy(yt_sb[:, 4:6, :], ytq[0][:])
                    nc.scalar.copy(yt_sb[:, 6:8, :], ytq[1][:])

                    # zB: qc 4-7 contributions for cols 126-255, fused
                    # add-evac onto the qc 0-3 partial already in z
                    for ih in range(2):
                        zb = ps2.tile([128, HO - ZCUT], F32, tag="ps2",
                                      name=f"ps2B_{ih}")
                        for qc in range(4, 8):
                            p2mm(zb[:], qc, ih, slice(ZCUT, HO),
                                 qc == 4, qc == 7)
                        nc.vector.scalar_tensor_tensor(
                            out=z[:, ih, ZCUT:HO],
                            in0=zb[:],
                            scalar=1.0,
                            in1=z[:, ih, ZCUT:HO],
                            op0=mybir.AluOpType.mult,
                            op1=mybir.AluOpType.add,
                        )
                    nc.sync.dma_start(out=out[:, img, :, ZCUT:HO],
                                      in_=z[:, :, ZCUT:HO])
    nc.compile()
    return nc


_GRAPH = None


def _get_graph():
    global _GRAPH
    if _GRAPH is None:
        _GRAPH = _build_graph()
    return _GRAPH


def run(x, **spmd_kwargs):
    x = np.ascontiguousarray(np.asarray(x, dtype=np.float32))
    assert x.shape == (B, C, H, W)
    nc = _get_graph()
    Tt = build_T().T  # [1024, 256] f32
    tb_host = np.ascontiguousarray(
        Tt.reshape(8, 128, HO).transpose(1, 0, 2)
    ).astype(ml_dtypes.bfloat16)
    eye_host = np.eye(128, dtype=ml_dtypes.bfloat16)
    per_core = B // N_CORES
    in_maps = [
        {
            "x": x[i * per_core : (i + 1) * per_core].reshape(IMGS, H, W),
            "tb": tb_host,
            "eye": eye_host,
        }
        for i in range(N_CORES)
    ]
    res = run_bass_kernel_spmd(nc, in_maps, core_ids=list(range(N_CORES)), **spmd_kwargs)
    outs = []
    for r in res.results:
        o = r["out"].transpose(1, 2, 0, 3).reshape(IMGS, 2 * 128, HO)
        outs.append(o.reshape(per_core, C, HO, HO))
    return np.concatenate(outs, axis=0), res


def kernel(x):
    out, _ = run(x)
    return out
